# revision 22
# baseline (speedup 1.0000x reference)
"""DecoderRNN (GRU + embedding + vocab projection) Bass kernel for 8 trn2 cores.

Model (per reference):
  toks = [2, x[0..S-2]]                          (teacher forcing, S=64, B=64)
  e_s  = relu(emb[toks_s])                       (E=512, padding row 0 = 0)
  GRU: r = sig(e@Wir^T + b_ir + h@Whr^T + b_hr)
       z = sig(e@Wiz^T + b_iz + h@Whz^T + b_hz)
       n = tanh(e@Win^T + b_in + r*(h@Whn^T + b_hn))
       h' = (1-z)*n + z*h                        (H=1024)
  logits_s = h_s @ Wout^T + b_out                (V=32000)
  out = logits.transpose(1,0,2)[None]            -> (1, B, S, V) f32

Distribution (v2): the GRU hidden dim is split 8 ways — core c owns h
columns [128c, 128c+128) and computes only its slice of the gates each
step (3072 PE cycles vs 24576 for the full recurrence), in TRANSPOSED
orientation [h_col, batch] so no per-step PE transpose is needed. The 8
h' slices are then all-gathered (HBM collective) so every core has the
full h for the next step and for its vocab-split output projection
(V/8 = 4000 cols/core), whose matmuls are interleaved into the step
loop to fill the PE during exchanges. The input-side GRU matmuls
(e@W_ih^T) are batched over all 64 steps upfront (teacher forcing
makes all tokens known) at full 128-wide PE efficiency. Logits are
written fp16 (halving the HBM write) and converted to f32 on host.

Layouts:
  hT slot s   [128, 8, 64] fp16: [p, k, b] = h_s[b, 128k+p]  (matmul rhs)
  h_c         [128, 64]    f32 : [j, b]    = h[b, 128c+j]    (own slice)
  w_gh        [128, 8, 3, 128] fp16: [p, k, g, j] = W_hh[gH+128c+j, 128k+p]
  gi store    [128, 3, SB] fp16: [j, g, 64s+b] = (e_s @ W_ih^T + bias)[b, gH+128c+j]
  hs pair t   [128, 8, 128] fp16: proj lhsT (tokens 128t..128t+127)
"""

import sys

sys.path.insert(0, "/opt/trn_rl_repo")

import numpy as np

import concourse.bass as bass
import concourse.bacc as bacc
import concourse.mybir as mybir
import concourse.tile as tile
from concourse.bass_utils import run_bass_kernel_spmd

FP16 = mybir.dt.float16
F32 = mybir.dt.float32
I32 = mybir.dt.int32

S, B, H, E, V = 64, 64, 1024, 512, 32000
NCORES = 8
VC = V // NCORES          # 4000 vocab cols per core
SB = S * B                # 4096
KH = H // 128             # 8 h k-chunks
KE = E // 128             # 4 e k-chunks
HC = 128                  # h cols owned per core
NN = 8                    # output n-chunks per core
NW = VC // NN             # 500 cols per n-chunk
NT = SB // 128            # 32 token tiles (= proj row tiles)
LEAD = 4                  # gi tiles computed ahead of the recurrence
RG = [list(range(NCORES))]

_CACHE = {}


def _build_probe():
    """Tiny NEFF: exchange logical rank ids over the XOR remote-DMA mesh.

    Validates the SWDGE remote-DMA path and measures sigma_r(d) = logical
    rank of the peer each core reaches at XOR slot d (the driver may
    permute logical rank -> physical TPB, and rdests XOR physical ids)."""
    if "probe" in _CACHE:
        return _CACHE["probe"]
    nc = bacc.Bacc("TRN2", target_bir_lowering=False, debug=False,
                   num_devices=NCORES)
    myid_d = nc.dram_tensor("myid", [128, 8], I32, kind="ExternalInput").ap()
    out_d = nc.dram_tensor("ids", [128, 8], I32, kind="ExternalOutput").ap()
    with tile.TileContext(nc) as tc:
        with tc.tile_pool(name="sb", bufs=1) as pb:
            rsem = nc.alloc_semaphore("xrsem")
            lsem = nc.alloc_semaphore("xlsem")
            myid = pb.tile([128, 8], I32, name="myid")
            ids = pb.tile([128, 8], I32, name="ids")
            nc.sync.dma_start(out=myid[:], in_=myid_d[:])
            nc.vector.tensor_copy(out=ids[:, 0:1], in_=myid[:, 0:1])
            nc.gpsimd.bir_kernel_barrier_wait(RG)
            for d in range(1, 8):
                rdests = [None] * 8
                rdests[d] = (0, d)
                nc.gpsimd.remote_dma_broadcast(
                    ids[:, d:d + 1], myid[:, 0:1],
                    rsem, lsem, rdests=rdests)
            nc.gpsimd.trigger_dma(count=None)
            nc.sync.dma_start(out=out_d[:], in_=ids[:])._wait_ge(rsem, 14)
    nc.compile()
    _CACHE["probe"] = nc
    return nc


def _discover_perms():
    """Run the probe once; returns sigma[r, d] or None if rdma unusable."""
    if "perms" in _CACHE:
        return _CACHE["perms"]
    perms = None
    try:
        nc = _build_probe()
        in_maps = [dict(myid=np.full((128, 8), c, dtype=np.int32))
                   for c in range(NCORES)]
        res = run_bass_kernel_spmd(nc, in_maps, list(range(NCORES)))
        sigma = np.stack([res.results[c]["ids"][0] for c in range(NCORES)])
        ok = all(sorted(sigma[r].tolist()) == list(range(NCORES))
                 and sigma[r][0] == r for r in range(NCORES))
        if ok:
            perms = sigma
    except Exception:
        perms = None
    _CACHE["perms"] = perms
    return perms


def _build(n_steps=S, dbg=False, xch="cc"):
    key = ("nc", n_steps, dbg, xch)
    if key in _CACHE:
        return _CACHE[key]

    nc = bacc.Bacc("TRN2", target_bir_lowering=False, debug=False,
                   num_devices=NCORES)

    def din(name, shape, dt):
        return nc.dram_tensor(name, shape, dt, kind="ExternalInput").ap()

    emb_d = din("emb_t", [V, E], FP16)
    idx_d = din("idx", [128, NT], I32)
    w_gh_d = din("w_gh", [128, KH, 3, HC], FP16)
    w_gi_d = din("w_gi", [128, KE, 3, HC], FP16)
    bias_gi_d = din("bias_gi", [128, 3, 128], F32)
    bias_hn_d = din("bias_hn", [128, 64], F32)
    hT0_d = din("hT0", [128, KH, 64], FP16)
    h0c_d = din("h0c", [128, 64], F32)
    w_outT_d = din("w_outT", [128, KH, VC], FP16)
    b_out_d = din("b_out_bc", [128, VC], FP16)
    out_d = nc.dram_tensor("out", [SB, VC], FP16, kind="ExternalOutput").ap()
    if dbg:
        dbg_gi_d = nc.dram_tensor("dbg_gi", [128, 3, SB], FP16,
                                  kind="ExternalOutput").ap()
        dbg_hT1_d = nc.dram_tensor("dbg_hT1", [128, KH, 64], FP16,
                                   kind="ExternalOutput").ap()
        dbg_rz0_d = nc.dram_tensor("dbg_rz0", [128, 2, 64], F32,
                                   kind="ExternalOutput").ap()
        dbg_hc1_d = nc.dram_tensor("dbg_hc1", [128, 64], F32,
                                   kind="ExternalOutput").ap()
        dbg_psg0_d = nc.dram_tensor("dbg_psg0", [128, 3, 64], F32,
                                    kind="ExternalOutput").ap()

    n_pairs = (n_steps + 1) // 2
    n_tiles = min(NT, n_pairs)

    with tile.TileContext(nc) as tc:
        with tc.tile_pool(name="const", bufs=1) as pc, \
             tc.tile_pool(name="roll", bufs=1) as pr, \
             tc.tile_pool(name="psum", bufs=1, space="PSUM") as pp, \
             tc.tile_pool(name="dram", bufs=1, space="DRAM") as pd:

            # ---- constants in SBUF
            w_gh = pc.tile([128, KH, 3, HC], FP16, name="w_gh")
            w_gi = pc.tile([128, KE, 3, HC], FP16, name="w_gi")
            bias_gi = pc.tile([128, 3, 128], F32, name="bias_gi")
            bias_hn = pc.tile([128, 64], F32, name="bias_hn")
            w_outT = pc.tile([128, KH, VC], FP16, name="w_outT")
            b_out = pc.tile([128, VC], FP16, name="b_out")
            idx = pc.tile([128, NT], I32, name="idx")
            gi = pc.tile([128, 3, SB], FP16, name="gi")

            for t, d in [(w_gh, w_gh_d), (w_gi, w_gi_d), (bias_gi, bias_gi_d),
                         (bias_hn, bias_hn_d), (w_outT, w_outT_d),
                         (b_out, b_out_d), (idx, idx_d)]:
                nc.sync.dma_start(out=t[:], in_=d[:])

            # ---- embedding gather pipeline (tokens 128g..128g+127 -> eT)
            def gather_tile(g):
                er = pr.tile([128, E], FP16, name=f"er{g}", tag="er", bufs=3)
                nc.gpsimd.indirect_dma_start(
                    out=er[:], out_offset=None,
                    in_=emb_d[:],
                    in_offset=bass.IndirectOffsetOnAxis(ap=idx[:, g:g + 1], axis=0),
                )
                eT = pr.tile([128, KE, 128], FP16, name=f"eT{g}", tag="eT", bufs=8)
                nc.sync.dma_start_transpose(out=eT[:], in_=er[:])
                return eT

            eT_w = {g: gather_tile(g) for g in range(min(LEAD + 2, n_tiles))}

            # ---- batched input projection: gi tile g covers tokens of
            # steps 2g, 2g+1 (128 tokens), all 3 gates, own 128 j-cols.
            def gi_tile(g):
                ps = pp.tile([128, 3, 128], F32, name=f"psgi{g}", tag="psgi",
                             bufs=2)
                eT = eT_w.pop(g)
                for gg in range(3):
                    for e in range(KE):
                        nc.tensor.matmul(
                            out=ps[:, gg, :], lhsT=w_gi[:, e, gg, :],
                            rhs=eT[:, e, :], start=(e == 0), stop=(e == KE - 1),
                            skip_group_check=True)
                nc.vector.tensor_tensor(
                    out=gi[:, :, 128 * g:128 * (g + 1)], in0=ps[:],
                    in1=bias_gi[:], op=mybir.AluOpType.add)

            for g in range(min(LEAD, n_tiles)):
                gi_tile(g)

            # ---- initial state
            hT = pr.tile([128, KH, 64], FP16, name="hT_init", tag="hT", bufs=4)
            h_c = pr.tile([128, 64], F32, name="hc_init", tag="hc", bufs=2)
            nc.sync.dma_start(out=hT[:], in_=hT0_d[:])
            nc.sync.dma_start(out=h_c[:], in_=h0c_d[:])

            hs_w = {}     # proj-ready lhsT tiles: t -> [128, KH, 128] fp16

            if xch == "rdma":
                rsem = nc.alloc_semaphore("hx_rsem")
                lsem = nc.alloc_semaphore("hx_lsem")
                nc.gpsimd.bir_kernel_barrier_wait(RG)

            jobs = [(t, nn) for t in range(n_tiles) for nn in range(NN)]
            jp = 0

            def emit_job(t, nn):
                ps_o = pp.tile([128, NW], F32, name=f"pso{t}_{nn}", tag="pso",
                               bufs=3)
                hst = hs_w[t]
                for k in range(KH):
                    nc.tensor.matmul(
                        out=ps_o[:], lhsT=hst[:, k, :],
                        rhs=w_outT[:, k, nn * NW:(nn + 1) * NW],
                        start=(k == 0), stop=(k == KH - 1),
                        skip_group_check=True)
                ob = pr.tile([128, NW], FP16, name=f"ob{t}_{nn}", tag="ob",
                             bufs=4)
                nc.vector.tensor_tensor(
                    out=ob[:], in0=ps_o[:], in1=b_out[:, nn * NW:(nn + 1) * NW],
                    op=mybir.AluOpType.add)
                nc.sync.dma_start(
                    out=out_d[t * 128:(t + 1) * 128, nn * NW:(nn + 1) * NW],
                    in_=ob[:])

            for s in range(n_steps):
                # ---- own-slice gate matmuls: [128 j, 64 b] per gate
                ps_g = pp.tile([128, 3, 64], F32, name=f"psg{s}", tag="psg",
                               bufs=2)
                first_mm = True
                for g in range(3):
                    for k in range(KH):
                        mm = nc.tensor.matmul(
                            out=ps_g[:, g, :], lhsT=w_gh[:, k, g, :],
                            rhs=hT[:, k, :], start=(k == 0), stop=(k == KH - 1),
                            skip_group_check=True)
                        if first_mm and xch == "rdma" and s > 0:
                            mm._wait_ge(rsem, 14 * s)
                        first_mm = False

                # ---- interleaved PE work while the gate chain runs:
                # next gi tile + ready output jobs
                if s % 2 == 0:
                    g_next = s // 2 + LEAD
                    if g_next < n_tiles:
                        gi_tile(g_next)
                    g_pre = s // 2 + LEAD + 2
                    if g_pre < n_tiles:
                        eT_w[g_pre] = gather_tile(g_pre)
                if s >= 2:
                    for _ in range(5):
                        if jp < len(jobs) and 2 * jobs[jp][0] + 2 <= s:
                            emit_job(*jobs[jp])
                            jp += 1

                if dbg and s == 0:
                    psg_sb = pr.tile([128, 3, 64], F32, name="psg_sb")
                    nc.vector.tensor_copy(out=psg_sb[:], in_=ps_g[:])
                    nc.sync.dma_start(out=dbg_psg0_d[:], in_=psg_sb[:])

                # ---- gate chain (DVE/ACT), all [128 j, *, 64 b]
                t_rz = pr.tile([128, 2, 64], F32, name=f"trz{s}", tag="trz",
                               bufs=2)
                nc.vector.tensor_tensor(
                    out=t_rz[:], in0=ps_g[:, 0:2, :],
                    in1=gi[:, 0:2, 64 * s:64 * s + 64], op=mybir.AluOpType.add)
                rz = pr.tile([128, 2, 64], F32, name=f"rz{s}", tag="rz", bufs=2)
                nc.scalar.activation(out=rz[:], in_=t_rz[:],
                                     func=mybir.ActivationFunctionType.Sigmoid)
                tn = pr.tile([128, 64], F32, name=f"tn{s}", tag="tn", bufs=2)
                nc.vector.tensor_tensor(out=tn[:], in0=ps_g[:, 2, :],
                                        in1=bias_hn[:], op=mybir.AluOpType.add)
                nc.vector.tensor_tensor(out=tn[:], in0=rz[:, 0, :], in1=tn[:],
                                        op=mybir.AluOpType.mult)
                nc.vector.tensor_tensor(
                    out=tn[:], in0=tn[:], in1=gi[:, 2, 64 * s:64 * s + 64],
                    op=mybir.AluOpType.add)
                n_sb = pr.tile([128, 64], F32, name=f"n{s}", tag="n", bufs=2)
                nc.scalar.activation(out=n_sb[:], in_=tn[:],
                                     func=mybir.ActivationFunctionType.Tanh)
                d_sb = pr.tile([128, 64], F32, name=f"d{s}", tag="d", bufs=2)
                nc.vector.tensor_tensor(out=d_sb[:], in0=h_c[:], in1=n_sb[:],
                                        op=mybir.AluOpType.subtract)
                nc.vector.tensor_tensor(out=d_sb[:], in0=rz[:, 1, :],
                                        in1=d_sb[:], op=mybir.AluOpType.mult)
                h_c = pr.tile([128, 64], F32, name=f"hc{s}", tag="hc", bufs=2)
                nc.vector.tensor_tensor(out=h_c[:], in0=n_sb[:], in1=d_sb[:],
                                        op=mybir.AluOpType.add)
                if xch == "cc":
                    # ---- all-gather h(s+1): 8 x [128, 64] -> [1024, 64]
                    h16 = pr.tile([128, 64], FP16, name=f"h16_{s}", tag="h16",
                                  bufs=3)
                    nc.vector.tensor_copy(out=h16[:], in_=h_c[:])
                    inb = pd.tile([128, 64], FP16, name=f"inb{s}", tag="inb",
                                  bufs=3)
                    outb = pd.tile([NCORES * 128, 64], FP16, name=f"outb{s}",
                                   tag="outb", bufs=3)
                    nc.sync.dma_start(out=inb[:], in_=h16[:])
                    nc.gpsimd.collective_compute(
                        "AllGather", mybir.AluOpType.bypass, replica_groups=RG,
                        ins=[inb[:]], outs=[outb[:]])
                    hT = pr.tile([128, KH, 64], FP16, name=f"hT{s}", tag="hT",
                                 bufs=4)
                    nc.sync.dma_start(
                        out=hT[:],
                        in_=outb[:].rearrange("(c p) b -> p c b", p=128))
                else:
                    # ---- XOR-mesh exchange: each peer's slot d receives my
                    # chunk; my slot d receives peer (phys XOR d)'s chunk.
                    hT = pr.tile([128, KH, 64], FP16, name=f"hT{s}", tag="hT",
                                 bufs=4)
                    nc.vector.tensor_copy(out=hT[:, 0, :], in_=h_c[:])
                    for dd in range(1, 8):
                        rdests = [None] * 8
                        rdests[dd] = (0, dd)
                        p = nc.gpsimd.remote_dma_broadcast(
                            hT[:, dd, :], hT[:, 0, :],
                            rsem, lsem, rdests=rdests)
                        if dd == 1 and s >= 2:
                            p._wait_ge(lsem, 112 * (s - 1))
                    nc.gpsimd.trigger_dma(count=None)

                if dbg and s == 0:
                    nc.sync.dma_start(out=dbg_rz0_d[:], in_=rz[:])
                    nc.sync.dma_start(out=dbg_hc1_d[:], in_=h_c[:])
                    nc.sync.dma_start(out=dbg_hT1_d[:], in_=hT[:])

                # ---- stage into the proj lhsT pair tile
                t_pair, half = s // 2, s % 2
                if half == 0:
                    hs_w[t_pair] = pr.tile([128, KH, 128], FP16,
                                           name=f"hs{t_pair}", tag="hs", bufs=4)
                cp = nc.vector.tensor_copy(
                    out=hs_w[t_pair][:, :, 64 * half:64 * half + 64],
                    in_=hT[:])
                if xch == "rdma":
                    cp._wait_ge(rsem, 14 * (s + 1))

            # ---- drain remaining output jobs
            while jp < len(jobs):
                emit_job(*jobs[jp])
                jp += 1

            if dbg:
                nc.sync.dma_start(out=dbg_gi_d[:], in_=gi[:])

    nc.compile()
    _CACHE[key] = nc
    return nc


def _prep_in_maps(x, hidden, emb, w_ih, w_hh, b_ih, b_hh, w_out, b_out,
                  perms=None):
    f16, f32 = np.float16, np.float32

    toks = np.concatenate([np.full((1, B), 2, dtype=np.int64),
                           np.asarray(x)[:-1].astype(np.int64)], axis=0)
    t_flat = toks.reshape(SB).astype(np.int32)
    idx = np.ascontiguousarray(t_flat.reshape(NT, 128).T)        # [128, 32]

    emb_t = np.asarray(emb, dtype=f32).copy()
    emb_t[0] = 0.0
    emb_t = np.maximum(emb_t, 0.0).astype(f16)                    # relu folded

    w_hh = np.asarray(w_hh, dtype=f32)                            # [3H, H]
    w_ih = np.asarray(w_ih, dtype=f32)                            # [3H, E]
    b_ih = np.asarray(b_ih, dtype=f32)
    b_hh = np.asarray(b_hh, dtype=f32)

    h0 = np.asarray(hidden, dtype=f32)[0]                         # [B, H]
    # hT0[p, k, b] = h0[b, 128k + p]
    hT0 = np.ascontiguousarray(
        h0.T.reshape(KH, 128, B).transpose(1, 0, 2)).astype(f16)

    w_out = np.asarray(w_out, dtype=f32)
    b_out = np.asarray(b_out, dtype=f32)

    shared = dict(emb_t=emb_t, idx=idx)

    # gate-major views [3, H(rows j), K]
    wg_hh = w_hh.reshape(3, H, H)
    wg_ih = w_ih.reshape(3, H, E)
    bg_i = b_ih.reshape(3, H)
    bg_h = b_hh.reshape(3, H)

    in_maps = []
    for c in range(NCORES):
        perm = list(perms[c]) if perms is not None else list(range(KH))
        js = slice(HC * c, HC * (c + 1))
        # w_gh[p, k, g, j] = W_hh[g*H + 128c + j, 128*perm[k] + p]
        w_gh = np.ascontiguousarray(
            wg_hh[:, js, :].reshape(3, HC, KH, 128)[:, :, perm, :]
            .transpose(3, 2, 0, 1)).astype(f16)
        w_gi = np.ascontiguousarray(
            wg_ih[:, js, :].reshape(3, HC, KE, 128)
            .transpose(3, 2, 0, 1)).astype(f16)
        # bias folded into gi: r,z get b_ih+b_hh; n gets b_ih only
        bg = np.stack([bg_i[0, js] + bg_h[0, js],
                       bg_i[1, js] + bg_h[1, js],
                       bg_i[2, js]], axis=0)                      # [3, 128]
        bias_gi = np.ascontiguousarray(
            np.broadcast_to(bg.T[:, :, None], (HC, 3, 128))).astype(f32)
        bias_hn = np.ascontiguousarray(
            np.broadcast_to(bg_h[2, js][:, None], (HC, 64))).astype(f32)
        h0c = np.ascontiguousarray(h0[:, js].T).astype(f32)       # [128, 64]

        vs = slice(c * VC, (c + 1) * VC)
        w_outT = np.ascontiguousarray(
            w_out[vs].T.reshape(KH, 128, VC)[perm]
            .transpose(1, 0, 2)).astype(f16)
        b_out_bc = np.ascontiguousarray(
            np.broadcast_to(b_out[vs], (128, VC))).astype(f16)
        hT0_c = np.ascontiguousarray(hT0[:, perm, :])
        in_maps.append(dict(shared, w_gh=w_gh, w_gi=w_gi, bias_gi=bias_gi,
                            bias_hn=bias_hn, h0c=h0c, w_outT=w_outT,
                            b_out_bc=b_out_bc, hT0=hT0_c))
    return in_maps


def _assemble(results):
    full = np.concatenate(
        [r["out"].astype(np.float32).reshape(S, B, VC) for r in results],
        axis=2)                                                   # (S, B, V)
    return np.ascontiguousarray(full.transpose(1, 0, 2)[None])


def _run(trace=False, tmpdir=None, xch=None, **inputs):
    if xch is None:
        xch = "rdma" if _discover_perms() is not None else "cc"
    perms = _discover_perms() if xch == "rdma" else None
    nc = _build(xch=xch)
    in_maps = _prep_in_maps(**inputs, perms=perms)
    res = run_bass_kernel_spmd(nc, in_maps, list(range(NCORES)),
                               trace=trace, tmpdir=tmpdir)
    return _assemble(res.results), res


def kernel(**inputs) -> np.ndarray:
    out, _ = _run(**inputs)
    return out


if __name__ == "__main__":
    rng = np.random.default_rng(0)
    ins = dict(
        x=rng.integers(0, V, (S, B)).astype(np.int32),
        hidden=rng.standard_normal((1, B, H)).astype(np.float32),
        emb=rng.standard_normal((V, E)).astype(np.float32),
        w_ih=rng.uniform(-1 / 32, 1 / 32, (3 * H, E)).astype(np.float32),
        w_hh=rng.uniform(-1 / 32, 1 / 32, (3 * H, H)).astype(np.float32),
        b_ih=rng.uniform(-1 / 32, 1 / 32, (3 * H,)).astype(np.float32),
        b_hh=rng.uniform(-1 / 32, 1 / 32, (3 * H,)).astype(np.float32),
        w_out=rng.uniform(-1 / 32, 1 / 32, (V, H)).astype(np.float32),
        b_out=rng.uniform(-1 / 32, 1 / 32, (V,)).astype(np.float32),
    )
    out = kernel(**ins)
    print("out", out.shape, out.dtype, float(np.abs(out).max()))
